# revision 27
# baseline (speedup 1.0000x reference)
import sys

sys.path.insert(0, "/opt/trn_rl_repo")

import numpy as np
import ml_dtypes

import concourse.bass as bass
import concourse.bacc as bacc
import concourse.mybir as mybir
from concourse.tile import TileContext
from concourse.bass_utils import run_bass_kernel_spmd

P = 128          # partitions
BT = 512         # batch-tile (free dim) per matmul / PSUM bank
G = 4            # batch groups packed into 128 partitions for the GRU
NCORES = 8
B, S, H, A = 131072, 256, 512, 32
BC = B // NCORES           # 16384 rows per core
MACRO = G * BT             # 2048 rows per GRU macro-tile
NM = BC // MACRO           # 8 macro-tiles per core
NP = NM // 2               # 4 macro-pairs

FP32 = mybir.dt.float32
BF16 = mybir.dt.bfloat16
BF = ml_dtypes.bfloat16
AF = mybir.ActivationFunctionType
OP = mybir.AluOpType

_CACHE = {}


def _build(nsteps: int) -> bass.Bass:
    nc = bacc.Bacc("TRN2", target_bir_lowering=False, debug=False,
                   num_devices=NCORES)

    xT = nc.dram_tensor("xT", [S, BC], BF16, kind="ExternalInput")
    w1t = nc.dram_tensor("w1t", [S, H], BF16, kind="ExternalInput")
    w2t = nc.dram_tensor("w2t", [H, H], BF16, kind="ExternalInput")
    wmt = nc.dram_tensor("wmt", [H, A], BF16, kind="ExternalInput")
    b1d = nc.dram_tensor("b1d", [P, 4], FP32, kind="ExternalInput")
    b2d = nc.dram_tensor("b2d", [P, 4], FP32, kind="ExternalInput")
    bmd = nc.dram_tensor("bmd", [A, 1], FP32, kind="ExternalInput")
    # augmented wp->gate mats: rows 0..7 = kron(I4, w_ih_x.T), rows 8..31
    # zero, row 32 = bias (partition offsets must be 32-aligned on HW)
    WR = 33
    lrid = nc.dram_tensor("lrid", [WR, P], BF16, kind="ExternalInput")
    luid = nc.dram_tensor("luid", [WR, P], BF16, kind="ExternalInput")
    lnid = nc.dram_tensor("lnid", [WR, P], BF16, kind="ExternalInput")
    lrhd = nc.dram_tensor("lrhd", [P, P], BF16, kind="ExternalInput")
    luhd = nc.dram_tensor("luhd", [P, P], BF16, kind="ExternalInput")
    lnhd = nc.dram_tensor("lnhd", [P, P], BF16, kind="ExternalInput")
    lwd = nc.dram_tensor("lwd", [P, 2 * G], BF16, kind="ExternalInput")
    bnhd = nc.dram_tensor("bnhd", [P, 1], FP32, kind="ExternalInput")
    bwd = nc.dram_tensor("bwd", [2 * G, 1], FP32, kind="ExternalInput")
    # packed: [pair, t, 2g+c, macro-half*512 + j]
    outT = nc.dram_tensor("outT", [NP, nsteps, 2 * G, 2 * BT], BF16,
                          kind="ExternalOutput")

    xv = xT.rearrange("(kb p) b -> p kb b", p=P)              # [128, 2, BC]

    with TileContext(nc) as tc:
        with (
            tc.tile_pool(name="const", bufs=1) as const,
            tc.tile_pool(name="state", bufs=1) as state,
            tc.tile_pool(name="xp", bufs=2) as xp,
            tc.tile_pool(name="h1p", bufs=2) as h1p,
            tc.tile_pool(name="h2p", bufs=2) as h2p,
            tc.tile_pool(name="gt", bufs=2) as gt,
            tc.tile_pool(name="pp", bufs=3, space="PSUM") as pp,
            tc.tile_pool(name="pw", bufs=2, space="PSUM") as pw,
        ):
            w1s = const.tile([P, 2, H], BF16)
            nc.sync.dma_start(w1s[:], w1t.rearrange("(kb p) f -> p kb f", p=P))
            w2s = const.tile([P, 4, H], BF16)
            nc.sync.dma_start(w2s[:], w2t.rearrange("(kb p) f -> p kb f", p=P))
            wms = const.tile([P, 4, A], BF16)
            nc.sync.dma_start(wms[:], wmt.rearrange("(kb p) f -> p kb f", p=P))
            b1s = const.tile([P, 4], FP32)
            nc.sync.dma_start(b1s[:], b1d[:])
            b2s = const.tile([P, 4], FP32)
            nc.sync.dma_start(b2s[:], b2d[:])
            bms = const.tile([A, 1], FP32)
            nc.sync.dma_start(bms[:], bmd[:])
            # GRU const tiles: allocated here, DMAs deferred until after the
            # first MLP units so the serial DMA-issue queue doesn't delay
            # the first matmul's X tile.
            lris = const.tile([WR, P], BF16)
            luis = const.tile([WR, P], BF16)
            lnis = const.tile([WR, P], BF16)
            lrhs = const.tile([P, P], BF16)
            luhs = const.tile([P, P], BF16)
            lnhs = const.tile([P, P], BF16)
            lws = const.tile([P, 2 * G], BF16)
            bnhs = const.tile([P, 1], FP32)
            bws = const.tile([2 * G, 1], FP32)

            def load_gru_consts():
                nc.sync.dma_start(lris[:], lrid[:])
                nc.sync.dma_start(luis[:], luid[:])
                nc.sync.dma_start(lnis[:], lnid[:])
                nc.sync.dma_start(lrhs[:], lrhd[:])
                nc.sync.dma_start(luhs[:], luhd[:])
                nc.sync.dma_start(lnhs[:], lnhd[:])
                nc.sync.dma_start(lws[:], lwd[:])
                nc.sync.dma_start(bnhs[:], bnhd[:])
                nc.sync.dma_start(bws[:], bwd[:])

            # per-pair persistent state
            Zb = []                       # [128, 2, 512] bf16, halves=macros
            WPa = []                      # [9, 1024] bf16 (row 8 == 1.0)
            WPb = []
            for p in range(NP):
                z = state.tile([P, 2, BT], BF16, tag=f"Z{p}")
                Zb.append(z)
                wa = state.tile([WR, 2 * BT], BF16, tag=f"WA{p}")
                nc.any.memset(wa[0:32, :], 0.0)
                nc.any.memset(wa[32:WR, :], 1.0)
                wb = state.tile([WR, 2 * BT], BF16, tag=f"WB{p}")
                nc.any.memset(wb[0:32, :], 0.0)
                nc.any.memset(wb[32:WR, :], 1.0)
                WPa.append(wa)
                WPb.append(wb)

            # ---------------- MLP encoder unit ----------------
            def mlp_unit(m, gp):         # macro m, group pair (2*gp, 2*gp+1)
                pr, mi = divmod(m, 2)
                c0 = m * MACRO + gp * 2 * BT
                X = xp.tile([P, 2, 2 * BT], BF16, tag="X")
                nc.sync.dma_start(X[:], xv[:, :, c0:c0 + 2 * BT])
                H1 = h1p.tile([P, 4, 2 * BT], BF16, tag="H1")
                for f in range(4):
                    ps = pp.tile([P, 2, BT], FP32, tag="pp")
                    for kb in range(2):
                        for g in range(2):
                            nc.tensor.matmul(
                                ps[:, g, :],
                                w1s[:, kb, f * P:(f + 1) * P],
                                X[:, kb, g * BT:(g + 1) * BT],
                                start=(kb == 0), stop=(kb == 1))
                    if f % 2 == 0:
                        nc.scalar.activation(H1[:, f, :], ps[:], AF.Relu,
                                             bias=b1s[:, f:f + 1])
                    else:
                        nc.vector.tensor_scalar(H1[:, f, :], ps[:],
                                                b1s[:, f:f + 1], 0.0,
                                                OP.add, OP.max)
                H2 = h2p.tile([P, 4, 2 * BT], BF16, tag="H2")
                for f in range(4):
                    ps = pp.tile([P, 2, BT], FP32, tag="pp")
                    for k in range(4):
                        for g in range(2):
                            nc.tensor.matmul(
                                ps[:, g, :],
                                w2s[:, k, f * P:(f + 1) * P],
                                H1[:, k, g * BT:(g + 1) * BT],
                                start=(k == 0), stop=(k == 3))
                    if f % 2 == 0:
                        nc.scalar.activation(H2[:, f, :], ps[:], AF.Relu,
                                             bias=b2s[:, f:f + 1])
                    else:
                        nc.vector.tensor_scalar(H2[:, f, :], ps[:],
                                                b2s[:, f:f + 1], 0.0,
                                                OP.add, OP.max)
                ps3 = pp.tile([A, 2, BT], FP32, tag="pp")
                for k in range(4):
                    for g in range(2):
                        nc.tensor.matmul(ps3[:, g, :], wms[:, k, :],
                                         H2[:, k, g * BT:(g + 1) * BT],
                                         start=(k == 0), stop=(k == 3))
                for g in range(2):
                    ga = 2 * gp + g
                    if g == 0:
                        nc.scalar.activation(
                            Zb[pr][ga * A:(ga + 1) * A, mi, :],
                            ps3[:, g, :], AF.Identity, bias=bms[:, :1])
                    else:
                        nc.vector.tensor_scalar(
                            Zb[pr][ga * A:(ga + 1) * A, mi, :],
                            ps3[:, g, :], bms[:, :1], None, OP.add)

            # ---------------- GRU step unit ----------------
            wp_cur = list(WPa)
            wp_nxt = list(WPb)

            def gru_step(t, p):
                if True:
                    Z = Zb[p]
                    WC = wp_cur[p]
                    WN = wp_nxt[p]
                    psRU0 = pp.tile([P, 2, BT], FP32, tag="pp")
                    psRU1 = pp.tile([P, 2, BT], FP32, tag="pp")
                    psRU = (psRU0, psRU1)
                    # R halves
                    for mi in range(2):
                        nc.tensor.matmul(psRU[mi][:, 0, :], lris[:],
                                         WC[:, mi * BT:(mi + 1) * BT],
                                         start=True, stop=False)
                    for mi in range(2):
                        nc.tensor.matmul(psRU[mi][:, 0, :], lrhs[:],
                                         Z[:, mi, :], start=False, stop=True)
                    # U halves
                    for mi in range(2):
                        nc.tensor.matmul(psRU[mi][:, 1, :], luis[:],
                                         WC[:, mi * BT:(mi + 1) * BT],
                                         start=True, stop=False)
                    for mi in range(2):
                        nc.tensor.matmul(psRU[mi][:, 1, :], luhs[:],
                                         Z[:, mi, :], start=False, stop=True)
                    NI2 = pp.tile([P, 2, BT], FP32, tag="pp")
                    for mi in range(2):
                        nc.tensor.matmul(NI2[:, mi, :], lnis[:],
                                         WC[:, mi * BT:(mi + 1) * BT],
                                         start=True, stop=True)
                    NH2 = pp.tile([P, 2, BT], FP32, tag="pp")
                    for mi in range(2):
                        nc.tensor.matmul(NH2[:, mi, :], lnhs[:],
                                         Z[:, mi, :], start=True, stop=True)

                    # gates: RUall dims [part, r/u, macro, col]
                    RU = gt.tile([P, 2, 2, BT], BF16, tag="RU")
                    for mi in range(2):
                        nc.scalar.activation(RU[:, :, mi, :], psRU[mi][:],
                                             AF.Sigmoid)
                    T1 = gt.tile([P, 2, BT], BF16, tag="T1")
                    nc.vector.scalar_tensor_tensor(
                        T1[:], NH2[:], bnhs[:, :1], RU[:, 0, :, :],
                        OP.add, OP.mult)
                    T2 = gt.tile([P, 2, BT], BF16, tag="T2")
                    nc.vector.tensor_tensor(T2[:], T1[:], NI2[:], OP.add)
                    N2 = gt.tile([P, 2, BT], BF16, tag="N2")
                    nc.scalar.activation(N2[:], T2[:], AF.Tanh)
                    ZD = gt.tile([P, 2, BT], BF16, tag="ZD")
                    nc.gpsimd.tensor_tensor(ZD[:], Z[:], N2[:], OP.subtract)
                    T3 = gt.tile([P, 2, BT], BF16, tag="T3")
                    nc.vector.tensor_tensor(T3[:], ZD[:], RU[:, 1, :, :],
                                            OP.mult)
                    nc.vector.tensor_tensor(Z[:], T3[:], N2[:], OP.add)

                    for mi in range(2):
                        psW = pw.tile([2 * G, BT], FP32, tag="pw")
                        nc.tensor.matmul(psW[:], lws[:], Z[:, mi, :],
                                         start=True, stop=True)
                        nc.vector.scalar_tensor_tensor(
                            WN[0:2 * G, mi * BT:(mi + 1) * BT], psW[:],
                            bws[:, :1], WC[0:2 * G, mi * BT:(mi + 1) * BT],
                            OP.add, OP.add)
                    nc.sync.dma_start(outT[p, t], WN[0:2 * G, :])
                    wp_cur[p], wp_nxt[p] = WN, WC

            # ---------------- schedule ----------------
            # Serial phases measure best: a single dense MLP block runs at
            # full clock until the firmware power-clamp, and the GRU runs
            # as 4-pair waves so the per-step serial chains hide each other.
            for m in range(NM):
                for gp in range(2):
                    mlp_unit(m, gp)
                if m == 0:
                    load_gru_consts()
            for t in range(nsteps):
                for p in range(NP):
                    gru_step(t, p)
    nc.compile()
    return nc


LAST_RESULT = None


def kernel(**inputs) -> np.ndarray:
    global LAST_RESULT
    x = np.asarray(inputs["x"], dtype=np.float32)
    W1 = np.asarray(inputs["W1"], np.float32)
    b1 = np.asarray(inputs["b1"], np.float32)
    W2 = np.asarray(inputs["W2"], np.float32)
    b2 = np.asarray(inputs["b2"], np.float32)
    Wm = np.asarray(inputs["Wm"], np.float32)
    bm = np.asarray(inputs["bm"], np.float32)
    w_ih = np.asarray(inputs["w_ih"], np.float32)
    w_hh = np.asarray(inputs["w_hh"], np.float32)
    b_ih = np.asarray(inputs["b_ih"], np.float32)
    b_hh = np.asarray(inputs["b_hh"], np.float32)
    Ww = np.asarray(inputs["Ww"], np.float32)
    bw = np.asarray(inputs["bw"], np.float32)
    T = int(inputs["pred_length"])

    I4 = np.eye(G, dtype=np.float32)

    def aug(w, brow):                    # [33, 128]: wp rows, zero pad, bias
        return np.concatenate(
            [np.kron(I4, w), np.zeros((24, P), np.float32), brow[None, :]],
            axis=0)

    common = {
        "w1t": np.ascontiguousarray(W1.T.astype(BF)),
        "w2t": np.ascontiguousarray(W2.T.astype(BF)),
        "wmt": np.ascontiguousarray(Wm.T.astype(BF)),
        "b1d": np.ascontiguousarray(b1.reshape(4, P).T),
        "b2d": np.ascontiguousarray(b2.reshape(4, P).T),
        "bmd": bm.reshape(A, 1).copy(),
        "lrid": np.ascontiguousarray(
            aug(w_ih[0:A].T, np.tile(b_ih[0:A] + b_hh[0:A], G)).astype(BF)),
        "luid": np.ascontiguousarray(
            aug(w_ih[A:2 * A].T,
                np.tile(b_ih[A:2 * A] + b_hh[A:2 * A], G)).astype(BF)),
        "lnid": np.ascontiguousarray(
            aug(w_ih[2 * A:3 * A].T,
                np.tile(b_ih[2 * A:3 * A], G)).astype(BF)),
        "lrhd": np.ascontiguousarray(np.kron(I4, w_hh[0:A].T).astype(BF)),
        "luhd": np.ascontiguousarray(np.kron(I4, w_hh[A:2 * A].T).astype(BF)),
        "lnhd": np.ascontiguousarray(
            np.kron(I4, w_hh[2 * A:3 * A].T).astype(BF)),
        "lwd": np.ascontiguousarray(np.kron(I4, Ww.T).astype(BF)),
        "bnhd": np.tile(b_hh[2 * A:3 * A], G).reshape(P, 1).copy(),
        "bwd": np.tile(bw, G).reshape(2 * G, 1).copy(),
    }
    xT = np.ascontiguousarray(x.astype(BF).T)          # [S, B] bf16
    in_maps = []
    for i in range(NCORES):
        m = dict(common)
        m["xT"] = np.ascontiguousarray(xT[:, i * BC:(i + 1) * BC])
        in_maps.append(m)

    if T not in _CACHE:
        _CACHE[T] = _build(T)
    nc = _CACHE[T]
    res = run_bass_kernel_spmd(nc, in_maps, core_ids=list(range(NCORES)))
    LAST_RESULT = res
    parts = []
    for i in range(NCORES):
        o = np.asarray(res.results[i]["outT"]).astype(np.float32)
        # [NP, T, 8, 1024] ; row = 2g+c, col = h*512+j ; macro = 2p+h
        o = o.reshape(NP, T, G, 2, 2, BT).transpose(0, 4, 2, 5, 1, 3)
        parts.append(o.reshape(BC, 2 * T))
    return np.ascontiguousarray(np.concatenate(parts, axis=0))


# revision 28
# speedup vs baseline: 1.1807x; 1.1807x over previous
import sys

sys.path.insert(0, "/opt/trn_rl_repo")

import numpy as np
import ml_dtypes

import concourse.bass as bass
import concourse.bacc as bacc
import concourse.mybir as mybir
from concourse.tile import TileContext
from concourse.bass_utils import run_bass_kernel_spmd

P = 128          # partitions
BT = 512         # batch-tile (free dim) per matmul / PSUM bank
G = 4            # batch groups packed into 128 partitions for the GRU
NCORES = 8
B, S, H, A = 131072, 256, 512, 32
BC = B // NCORES           # 16384 rows per core
MACRO = G * BT             # 2048 rows per GRU macro-tile
NM = BC // MACRO           # 8 macro-tiles per core
NP = NM // 2               # 4 macro-pairs

FP32 = mybir.dt.float32
BF16 = mybir.dt.bfloat16
BF = ml_dtypes.bfloat16
AF = mybir.ActivationFunctionType
OP = mybir.AluOpType

_CACHE = {}


def _build(nsteps: int) -> bass.Bass:
    nc = bacc.Bacc("TRN2", target_bir_lowering=False, debug=False,
                   num_devices=NCORES)

    xT = nc.dram_tensor("xT", [S, BC], BF16, kind="ExternalInput")
    w1t = nc.dram_tensor("w1t", [S, H], BF16, kind="ExternalInput")
    w2t = nc.dram_tensor("w2t", [H, H], BF16, kind="ExternalInput")
    wmt = nc.dram_tensor("wmt", [H, A], BF16, kind="ExternalInput")
    b1d = nc.dram_tensor("b1d", [P, 4], FP32, kind="ExternalInput")
    b2d = nc.dram_tensor("b2d", [P, 4], FP32, kind="ExternalInput")
    bmd = nc.dram_tensor("bmd", [A, 1], FP32, kind="ExternalInput")
    # augmented wp->gate mats: rows 0..7 = kron(I4, w_ih_x.T), rows 8..31
    # zero, row 32 = bias (partition offsets must be 32-aligned on HW)
    WR = 33
    lrid = nc.dram_tensor("lrid", [WR, P], BF16, kind="ExternalInput")
    luid = nc.dram_tensor("luid", [WR, P], BF16, kind="ExternalInput")
    lnid = nc.dram_tensor("lnid", [WR, P], BF16, kind="ExternalInput")
    lrhd = nc.dram_tensor("lrhd", [P, P], BF16, kind="ExternalInput")
    luhd = nc.dram_tensor("luhd", [P, P], BF16, kind="ExternalInput")
    lnhd = nc.dram_tensor("lnhd", [P, P], BF16, kind="ExternalInput")
    lwd = nc.dram_tensor("lwd", [P, 2 * G], BF16, kind="ExternalInput")
    bnhd = nc.dram_tensor("bnhd", [P, 1], FP32, kind="ExternalInput")
    bwd = nc.dram_tensor("bwd", [2 * G, 1], FP32, kind="ExternalInput")
    # packed: [pair, t, 2g+c, macro-half*512 + j]
    outT = nc.dram_tensor("outT", [NP, nsteps, 2 * G, 2 * BT], BF16,
                          kind="ExternalOutput")

    xv = xT.rearrange("(kb p) b -> p kb b", p=P)              # [128, 2, BC]

    with TileContext(nc) as tc:
        with (
            tc.tile_pool(name="const", bufs=1) as const,
            tc.tile_pool(name="state", bufs=1) as state,
            tc.tile_pool(name="xp", bufs=2) as xp,
            tc.tile_pool(name="h1p", bufs=2) as h1p,
            tc.tile_pool(name="h2p", bufs=2) as h2p,
            tc.tile_pool(name="gt", bufs=2) as gt,
            tc.tile_pool(name="pp", bufs=3, space="PSUM") as pp,
            tc.tile_pool(name="pw", bufs=2, space="PSUM") as pw,
        ):
            w1s = const.tile([P, 2, H], BF16)
            nc.sync.dma_start(w1s[:], w1t.rearrange("(kb p) f -> p kb f", p=P))
            w2s = const.tile([P, 4, H], BF16)
            nc.sync.dma_start(w2s[:], w2t.rearrange("(kb p) f -> p kb f", p=P))
            wms = const.tile([P, 4, A], BF16)
            nc.sync.dma_start(wms[:], wmt.rearrange("(kb p) f -> p kb f", p=P))
            b1s = const.tile([P, 4], FP32)
            nc.sync.dma_start(b1s[:], b1d[:])
            b2s = const.tile([P, 4], FP32)
            nc.sync.dma_start(b2s[:], b2d[:])
            bms = const.tile([A, 1], FP32)
            nc.sync.dma_start(bms[:], bmd[:])
            # GRU const tiles: allocated here, DMAs deferred until after the
            # first MLP units so the serial DMA-issue queue doesn't delay
            # the first matmul's X tile.
            lris = const.tile([WR, P], BF16)
            luis = const.tile([WR, P], BF16)
            lnis = const.tile([WR, P], BF16)
            lrhs = const.tile([P, P], BF16)
            luhs = const.tile([P, P], BF16)
            lnhs = const.tile([P, P], BF16)
            lws = const.tile([P, 2 * G], BF16)
            bnhs = const.tile([P, 1], FP32)
            bws = const.tile([2 * G, 1], FP32)

            def load_gru_consts():
                nc.sync.dma_start(lris[:], lrid[:])
                nc.sync.dma_start(luis[:], luid[:])
                nc.sync.dma_start(lnis[:], lnid[:])
                nc.sync.dma_start(lrhs[:], lrhd[:])
                nc.sync.dma_start(luhs[:], luhd[:])
                nc.sync.dma_start(lnhs[:], lnhd[:])
                nc.sync.dma_start(lws[:], lwd[:])
                nc.sync.dma_start(bnhs[:], bnhd[:])
                nc.sync.dma_start(bws[:], bwd[:])

            # per-pair persistent state
            Zb = []                       # [128, 2, 512] bf16, halves=macros
            WPa = []                      # [9, 1024] bf16 (row 8 == 1.0)
            WPb = []
            for p in range(NP):
                z = state.tile([P, 2, BT], BF16, tag=f"Z{p}")
                Zb.append(z)
                wa = state.tile([WR, 2 * BT], BF16, tag=f"WA{p}")
                nc.any.memset(wa[0:32, :], 0.0)
                nc.any.memset(wa[32:WR, :], 1.0)
                wb = state.tile([WR, 2 * BT], BF16, tag=f"WB{p}")
                nc.any.memset(wb[0:32, :], 0.0)
                nc.any.memset(wb[32:WR, :], 1.0)
                WPa.append(wa)
                WPb.append(wb)

            # ---------------- MLP encoder unit ----------------
            def mlp_unit(m, gp):         # macro m, group pair (2*gp, 2*gp+1)
                pr, mi = divmod(m, 2)
                c0 = m * MACRO + gp * 2 * BT
                X = xp.tile([P, 2, 2 * BT], BF16, tag="X")
                nc.sync.dma_start(X[:], xv[:, :, c0:c0 + 2 * BT])
                H1 = h1p.tile([P, 4, 2 * BT], BF16, tag="H1")
                for f in range(4):
                    ps = pp.tile([P, 2, BT], FP32, tag="pp")
                    for kb in range(2):
                        for g in range(2):
                            nc.tensor.matmul(
                                ps[:, g, :],
                                w1s[:, kb, f * P:(f + 1) * P],
                                X[:, kb, g * BT:(g + 1) * BT],
                                start=(kb == 0), stop=(kb == 1))
                    if f % 2 == 0:
                        nc.scalar.activation(H1[:, f, :], ps[:], AF.Relu,
                                             bias=b1s[:, f:f + 1])
                    else:
                        nc.vector.tensor_scalar(H1[:, f, :], ps[:],
                                                b1s[:, f:f + 1], 0.0,
                                                OP.add, OP.max)
                H2 = h2p.tile([P, 4, 2 * BT], BF16, tag="H2")
                for f in range(4):
                    ps = pp.tile([P, 2, BT], FP32, tag="pp")
                    for k in range(4):
                        for g in range(2):
                            nc.tensor.matmul(
                                ps[:, g, :],
                                w2s[:, k, f * P:(f + 1) * P],
                                H1[:, k, g * BT:(g + 1) * BT],
                                start=(k == 0), stop=(k == 3))
                    if f % 2 == 0:
                        nc.scalar.activation(H2[:, f, :], ps[:], AF.Relu,
                                             bias=b2s[:, f:f + 1])
                    else:
                        nc.vector.tensor_scalar(H2[:, f, :], ps[:],
                                                b2s[:, f:f + 1], 0.0,
                                                OP.add, OP.max)
                ps3 = pp.tile([A, 2, BT], FP32, tag="pp")
                for k in range(4):
                    for g in range(2):
                        nc.tensor.matmul(ps3[:, g, :], wms[:, k, :],
                                         H2[:, k, g * BT:(g + 1) * BT],
                                         start=(k == 0), stop=(k == 3))
                for g in range(2):
                    ga = 2 * gp + g
                    nc.scalar.activation(
                        Zb[pr][ga * A:(ga + 1) * A, mi, :],
                        ps3[:, g, :], AF.Identity, bias=bms[:, :1])

            # ---------------- GRU step unit ----------------
            wp_cur = list(WPa)
            wp_nxt = list(WPb)

            def gru_step(t, p):
                if True:
                    Z = Zb[p]
                    WC = wp_cur[p]
                    WN = wp_nxt[p]
                    psRU0 = pp.tile([P, 2, BT], FP32, tag="pp")
                    psRU1 = pp.tile([P, 2, BT], FP32, tag="pp")
                    psRU = (psRU0, psRU1)
                    # R halves
                    for mi in range(2):
                        nc.tensor.matmul(psRU[mi][:, 0, :], lris[:],
                                         WC[:, mi * BT:(mi + 1) * BT],
                                         start=True, stop=False)
                    for mi in range(2):
                        nc.tensor.matmul(psRU[mi][:, 0, :], lrhs[:],
                                         Z[:, mi, :], start=False, stop=True)
                    # U halves
                    for mi in range(2):
                        nc.tensor.matmul(psRU[mi][:, 1, :], luis[:],
                                         WC[:, mi * BT:(mi + 1) * BT],
                                         start=True, stop=False)
                    for mi in range(2):
                        nc.tensor.matmul(psRU[mi][:, 1, :], luhs[:],
                                         Z[:, mi, :], start=False, stop=True)
                    NI2 = pp.tile([P, 2, BT], FP32, tag="pp")
                    for mi in range(2):
                        nc.tensor.matmul(NI2[:, mi, :], lnis[:],
                                         WC[:, mi * BT:(mi + 1) * BT],
                                         start=True, stop=True)
                    NH2 = pp.tile([P, 2, BT], FP32, tag="pp")
                    for mi in range(2):
                        nc.tensor.matmul(NH2[:, mi, :], lnhs[:],
                                         Z[:, mi, :], start=True, stop=True)

                    # gates: RUall dims [part, r/u, macro, col]
                    RU = gt.tile([P, 2, 2, BT], BF16, tag="RU")
                    for mi in range(2):
                        nc.scalar.activation(RU[:, :, mi, :], psRU[mi][:],
                                             AF.Sigmoid)
                    T1 = gt.tile([P, 2, BT], BF16, tag="T1")
                    nc.vector.scalar_tensor_tensor(
                        T1[:], NH2[:], bnhs[:, :1], RU[:, 0, :, :],
                        OP.add, OP.mult)
                    T2 = gt.tile([P, 2, BT], BF16, tag="T2")
                    nc.vector.tensor_tensor(T2[:], T1[:], NI2[:], OP.add)
                    N2 = gt.tile([P, 2, BT], BF16, tag="N2")
                    nc.scalar.activation(N2[:], T2[:], AF.Tanh)
                    ZD = gt.tile([P, 2, BT], BF16, tag="ZD")
                    nc.gpsimd.tensor_tensor(ZD[:], Z[:], N2[:], OP.subtract)
                    T3 = gt.tile([P, 2, BT], BF16, tag="T3")
                    nc.vector.tensor_tensor(T3[:], ZD[:], RU[:, 1, :, :],
                                            OP.mult)
                    nc.vector.tensor_tensor(Z[:], T3[:], N2[:], OP.add)

                    for mi in range(2):
                        psW = pw.tile([2 * G, BT], FP32, tag="pw")
                        nc.tensor.matmul(psW[:], lws[:], Z[:, mi, :],
                                         start=True, stop=True)
                        nc.vector.scalar_tensor_tensor(
                            WN[0:2 * G, mi * BT:(mi + 1) * BT], psW[:],
                            bws[:, :1], WC[0:2 * G, mi * BT:(mi + 1) * BT],
                            OP.add, OP.add)
                    nc.sync.dma_start(outT[p, t], WN[0:2 * G, :])
                    wp_cur[p], wp_nxt[p] = WN, WC

            # ---------------- schedule ----------------
            # Serial phases measure best: a single dense MLP block runs at
            # full clock until the firmware power-clamp, and the GRU runs
            # as 4-pair waves so the per-step serial chains hide each other.
            for m in range(NM):
                for gp in range(2):
                    mlp_unit(m, gp)
                if m == 0:
                    load_gru_consts()
            for t in range(nsteps):
                for p in range(NP):
                    gru_step(t, p)
    nc.compile()
    return nc


LAST_RESULT = None


def kernel(**inputs) -> np.ndarray:
    global LAST_RESULT
    x = np.asarray(inputs["x"], dtype=np.float32)
    W1 = np.asarray(inputs["W1"], np.float32)
    b1 = np.asarray(inputs["b1"], np.float32)
    W2 = np.asarray(inputs["W2"], np.float32)
    b2 = np.asarray(inputs["b2"], np.float32)
    Wm = np.asarray(inputs["Wm"], np.float32)
    bm = np.asarray(inputs["bm"], np.float32)
    w_ih = np.asarray(inputs["w_ih"], np.float32)
    w_hh = np.asarray(inputs["w_hh"], np.float32)
    b_ih = np.asarray(inputs["b_ih"], np.float32)
    b_hh = np.asarray(inputs["b_hh"], np.float32)
    Ww = np.asarray(inputs["Ww"], np.float32)
    bw = np.asarray(inputs["bw"], np.float32)
    T = int(inputs["pred_length"])

    I4 = np.eye(G, dtype=np.float32)

    def aug(w, brow):                    # [33, 128]: wp rows, zero pad, bias
        return np.concatenate(
            [np.kron(I4, w), np.zeros((24, P), np.float32), brow[None, :]],
            axis=0)

    common = {
        "w1t": np.ascontiguousarray(W1.T.astype(BF)),
        "w2t": np.ascontiguousarray(W2.T.astype(BF)),
        "wmt": np.ascontiguousarray(Wm.T.astype(BF)),
        "b1d": np.ascontiguousarray(b1.reshape(4, P).T),
        "b2d": np.ascontiguousarray(b2.reshape(4, P).T),
        "bmd": bm.reshape(A, 1).copy(),
        "lrid": np.ascontiguousarray(
            aug(w_ih[0:A].T, np.tile(b_ih[0:A] + b_hh[0:A], G)).astype(BF)),
        "luid": np.ascontiguousarray(
            aug(w_ih[A:2 * A].T,
                np.tile(b_ih[A:2 * A] + b_hh[A:2 * A], G)).astype(BF)),
        "lnid": np.ascontiguousarray(
            aug(w_ih[2 * A:3 * A].T,
                np.tile(b_ih[2 * A:3 * A], G)).astype(BF)),
        "lrhd": np.ascontiguousarray(np.kron(I4, w_hh[0:A].T).astype(BF)),
        "luhd": np.ascontiguousarray(np.kron(I4, w_hh[A:2 * A].T).astype(BF)),
        "lnhd": np.ascontiguousarray(
            np.kron(I4, w_hh[2 * A:3 * A].T).astype(BF)),
        "lwd": np.ascontiguousarray(np.kron(I4, Ww.T).astype(BF)),
        "bnhd": np.tile(b_hh[2 * A:3 * A], G).reshape(P, 1).copy(),
        "bwd": np.tile(bw, G).reshape(2 * G, 1).copy(),
    }
    xT = np.ascontiguousarray(x.astype(BF).T)          # [S, B] bf16
    in_maps = []
    for i in range(NCORES):
        m = dict(common)
        m["xT"] = np.ascontiguousarray(xT[:, i * BC:(i + 1) * BC])
        in_maps.append(m)

    if T not in _CACHE:
        _CACHE[T] = _build(T)
    nc = _CACHE[T]
    res = run_bass_kernel_spmd(nc, in_maps, core_ids=list(range(NCORES)))
    LAST_RESULT = res
    parts = []
    for i in range(NCORES):
        o = np.asarray(res.results[i]["outT"]).astype(np.float32)
        # [NP, T, 8, 1024] ; row = 2g+c, col = h*512+j ; macro = 2p+h
        o = o.reshape(NP, T, G, 2, 2, BT).transpose(0, 4, 2, 5, 1, 3)
        parts.append(o.reshape(BC, 2 * T))
    return np.ascontiguousarray(np.concatenate(parts, axis=0))
